# revision 1
# baseline (speedup 1.0000x reference)
"""Trainium2 Bass kernel for nn_CTA_28381143891994.

Continuous-time-attention GRU scan:  per-step ODE (tanh MLP), sigmoid
attention over a fixed query, GRU state update.  Data-parallel over batch
across 8 NeuronCores (8 batch rows per core).

Key reformulations (host-side, exact):
  * score_t = z_t @ (Wk @ q) + bk@q           -- Wk matmul eliminated
  * c = dt*[ (sum_t a_t z_t) @ Wv + (sum_t a_t) bv ]  -- Wv matmul moved
    out of the scan (rank-1 accumulators r, s)
  * xg_t = x_t @ (Wp @ gru_K) + const         -- proj folded into gru_K
On-device everything is kept transposed (hidden on partitions, batch on
the free axis) so elementwise work uses all 128 lanes; per-step matmuls
run weights-stationary in fp16 (fp32 PSUM accumulation).
"""

import numpy as np

import concourse.bass as bass
import concourse.mybir as mybir
import concourse.tile as tile
from concourse.bass import ds
from concourse.bass_utils import run_bass_kernel_spmd
from concourse.tile import TileContext, ScopedClock

F16 = mybir.dt.float16
F32 = mybir.dt.float32
AF = mybir.ActivationFunctionType
OP = mybir.AluOpType

B, S, DIN, H = 64, 2048, 512, 512
NCORES = 8
BL = B // NCORES          # 8 batch rows per core
HC = H // 128             # 4 hidden chunks
G = 3 * H                 # gru gate width 1536
GJ = G // 128             # 12 gate chunks


# ---------------------------------------------------------------------------
# Workaround: this walrus build only accepts a single sync-wait per Drain
# instruction, and the butterfly all-engine barrier emits Drains with
# eq-waits.  Split the tail-drain waits one-per-Drain and use the
# sequencer-level (sem-only) barrier instead.
# ---------------------------------------------------------------------------
def _patched_drain_and_barrier(self, tick_clock, wait_clock):
    nc = self.nc
    d = nc.sync.drain()
    wait_clock.add_sem_waits(d.ins, ScopedClock({None: tick_clock.global_clock}))
    waits = list(d.ins.sync_info.on_wait)
    if len(waits) > 1:
        d.ins.sync_info.on_wait = waits[:1]
        rest = waits[1:]
        while rest:
            d2 = nc.sync.drain()
            d2.ins.sync_info = mybir.SyncInfo(on_wait=rest[:1], on_update=[])
            rest = rest[1:]
    nc.all_engine_barrier(sem_only=True)
    popped = nc._tile_sem_poison_stack.pop()
    assert popped is self._sem_poison
    nc.clear_and_free_semaphores(list(self.sems.allocated().values()))
    nc.all_engine_barrier(sem_only=True)


if not getattr(TileContext, "_cta_drain_patch", False):
    TileContext._drain_and_barrier = _patched_drain_and_barrier
    TileContext._cta_drain_patch = True


def _split_multi_waits(nc):
    """This walrus build encodes at most one sem-wait per instruction.
    Move extra waits onto same-engine NoOps placed just before the owner."""
    ctr = [0]

    def mk_wait_nop(engine, wait):
        ctr[0] += 1
        nop = mybir.InstNoOp(name=f"WSPL-{ctr[0]}", ins=[], outs=[], engine=engine)
        nop.sync_info = mybir.SyncInfo(on_wait=[wait], on_update=[])
        nc.register_instruction(nop, overwrite=True)
        return nop

    for f in nc.m.functions:
        for bb in f.blocks:
            out = []
            changed = False
            for inst in bb.instructions:
                si = inst.sync_info
                if si is not None and si.on_wait and len(si.on_wait) > 1:
                    waits = list(si.on_wait)
                    for w in waits[:-1]:
                        out.append(mk_wait_nop(inst.engine, w))
                    inst.sync_info = mybir.SyncInfo(
                        on_wait=waits[-1:], on_update=list(si.on_update)
                    )
                    changed = True
                out.append(inst)
            if changed:
                bb.instructions = out


# ---------------------------------------------------------------------------
# Program builder
# ---------------------------------------------------------------------------
def build_program(n_steps=S):
    """n_steps: total timesteps incl. t=0 (scan runs t=1..n_steps-1).
    Must be a multiple of 32 and >= 64."""
    assert n_steps % 32 == 0 and n_steps >= 64
    n_bodies = n_steps // 32          # first handled by prologue
    nslots = n_steps // 64            # phase-A 64-step slots

    nc = bass.Bass()
    xt = nc.dram_tensor("xt", [DIN, S, BL], F16, kind="ExternalInput")
    z0t = nc.dram_tensor("z0t", [128, HC * BL], F32, kind="ExternalInput")
    rw = nc.dram_tensor("rw", [128, HC, G], F16, kind="ExternalInput")
    wxg = nc.dram_tensor("wxg", [128, HC, G], F16, kind="ExternalInput")
    wv = nc.dram_tensor("wv", [128, HC, H], F16, kind="ExternalInput")
    w1 = nc.dram_tensor("w1", [128, HC, 32], F16, kind="ExternalInput")
    wkq = nc.dram_tensor("wkq", [128, HC, 1], F16, kind="ExternalInput")
    w2p = nc.dram_tensor("w2p", [128, 32], F16, kind="ExternalInput")
    w3p = nc.dram_tensor("w3p", [128, H], F16, kind="ExternalInput")
    w3r = nc.dram_tensor("w3r", [128, G], F16, kind="ExternalInput")
    eye = nc.dram_tensor("eye", [128, 128], F16, kind="ExternalInput")
    onesl = nc.dram_tensor("onesl", [128, 128], F16, kind="ExternalInput")
    constj = nc.dram_tensor("constj", [128, GJ], F32, kind="ExternalInput")
    dtcol = nc.dram_tensor("dtcol", [128, 1], F32, kind="ExternalInput")
    sca = nc.dram_tensor("sca", [1, 1], F32, kind="ExternalInput")
    bia = nc.dram_tensor("bia", [1, 1], F32, kind="ExternalInput")
    b1t = nc.dram_tensor("b1t", [32, 1], F32, kind="ExternalInput")
    b2t = nc.dram_tensor("b2t", [32, 1], F32, kind="ExternalInput")
    outt = nc.dram_tensor("outt", [128, HC * BL], F32, kind="ExternalOutput")
    import os as _os
    _dbg = _os.environ.get("CTA_DBG") == "1"
    if _dbg:
        zdbg1 = nc.dram_tensor("zdbg1", [128, HC * BL], F32, kind="ExternalOutput")
        zdbg2 = nc.dram_tensor("zdbg2", [128, HC * BL], F32, kind="ExternalOutput")

    with TileContext(nc) as tc:
        with (
            tc.tile_pool(name="wpool", bufs=1) as wp,
            tc.tile_pool(name="dram", bufs=1, space="DRAM") as dp,
        ):
            xgt = dp.tile([128, n_steps + 32, GJ * BL], F16)

            # ---- persistent SBUF ----
            rw_sb = wp.tile([128, HC, G], F16)
            nc.sync.dma_start(rw_sb[:], rw[:])
            wxg_sb = wp.tile([128, HC, G], F16)
            nc.sync.dma_start(wxg_sb[:], wxg[:])
            wv_sb = wp.tile([128, HC, H], F16)
            nc.sync.dma_start(wv_sb[:], wv[:])
            w1_sb = wp.tile([128, HC, 32], F16)
            nc.sync.dma_start(w1_sb[:], w1[:])
            wkq_sb = wp.tile([128, HC, 1], F16)
            nc.sync.dma_start(wkq_sb[:], wkq[:])
            w2_sb = wp.tile([128, 32], F16)
            nc.sync.dma_start(w2_sb[:], w2p[:])
            w3_sb = wp.tile([128, H], F16)
            nc.sync.dma_start(w3_sb[:], w3p[:])
            w3r_sb = wp.tile([128, G], F16)
            nc.sync.dma_start(w3r_sb[:], w3r[:])
            eye_sb = wp.tile([128, 128], F16)
            nc.sync.dma_start(eye_sb[:], eye[:])
            ones_sb = wp.tile([128, 128], F16)
            nc.sync.dma_start(ones_sb[:], onesl[:])
            constj_sb = wp.tile([128, GJ], F32)
            nc.sync.dma_start(constj_sb[:], constj[:])
            dt_sb = wp.tile([128, 1], F32)
            nc.sync.dma_start(dt_sb[:], dtcol[:])
            sca_sb = wp.tile([1, 1], F32)
            nc.sync.dma_start(sca_sb[:], sca[:])
            bia_sb = wp.tile([1, 1], F32)
            nc.sync.dma_start(bia_sb[:], bia[:])
            b1_sb = wp.tile([32, 1], F32)
            nc.sync.dma_start(b1_sb[:], b1t[:])
            b2_sb = wp.tile([32, 1], F32)
            nc.sync.dma_start(b2_sb[:], b2t[:])

            zA_f = wp.tile([128, HC * BL], F32)
            nc.sync.dma_start(zA_f[:], z0t[:])
            zA_h = wp.tile([128, HC * BL], F16)
            nc.vector.tensor_copy(zA_h[:], zA_f[:])
            zB_f = wp.tile([128, HC * BL], F32)
            zB_h = wp.tile([128, HC * BL], F16)
            zh_f = wp.tile([128, HC * BL], F32)
            rT = wp.tile([128, HC * BL], F32)
            nc.vector.memset(rT[:], 0.0)
            sRow = wp.tile([1, BL], F32)
            nc.vector.memset(sRow[:], 0.0)
            zro16 = wp.tile([128, HC * BL], F16)
            nc.vector.memset(zro16[:], 0.0)
            alphaF = wp.tile([1, BL], F32)
            alco = wp.tile([128, BL], F16)
            nc.vector.memset(alco[:], 0.0)
            h1T = wp.tile([128, BL], F16)
            nc.vector.memset(h1T[:], 0.0)
            h2T = wp.tile([128, BL], F16)
            nc.vector.memset(h2T[:], 0.0)
            S0 = wp.tile([128, 16, GJ * BL], F16)
            S1 = wp.tile([128, 16, GJ * BL], F16)
            zgrg = wp.tile([128, 2 * HC * BL], F32)
            t3 = wp.tile([128, HC * BL], F16)
            t4 = wp.tile([128, HC * BL], F16)
            candT = wp.tile([128, HC * BL], F32)
            wgT = wp.tile([128, HC * BL], F32)
            eT = wp.tile([128, HC * BL], F32)
            uT = wp.tile([128, HC * BL], F32)
            rtmp = wp.tile([128, HC * BL], F32)

            # ================= PHASE A: XGT precompute =================
            with (
                tc.tile_pool(name="pa", bufs=3) as pa,
                tc.tile_pool(name="pap", bufs=2, space="PSUM") as pap,
            ):
                for sl in range(nslots):
                    t0 = 64 * sl
                    xts = pa.tile([128, HC, 64, BL], F16, tag="xts")
                    for c in range(HC):
                        nc.sync.dma_start(
                            xts[:, c], xt[c * 128:(c + 1) * 128, t0:t0 + 64, :]
                        )
                    slot = pa.tile([128, 64, GJ, BL], F16, tag="slot")
                    for j in range(GJ):
                        pj = pap.tile([128, 64 * BL], F32, tag="pj")
                        for c in range(HC):
                            nc.tensor.matmul(
                                pj[:],
                                lhsT=wxg_sb[:, c, j * 128:(j + 1) * 128],
                                rhs=xts[:, c],
                                start=(c == 0),
                                stop=(c == HC - 1),
                            )
                        nc.vector.tensor_scalar(
                            slot[:, :, j, :],
                            pj.rearrange("p (t b) -> p t b", b=BL),
                            constj_sb[:, j:j + 1],
                            None,
                            OP.add,
                        )
                    nc.sync.dma_start(
                        xgt[:, t0:t0 + 64, :],
                        slot.rearrange("p t j b -> p t (j b)"),
                    )
                # zero-fill the 32-row lookahead pad past the last real step
                zpad = pa.tile([128, 32, GJ * BL], F16, tag="zpad")
                nc.vector.memset(zpad[:], 0.0)
                nc.sync.dma_start(xgt[:, n_steps:n_steps + 32, :], zpad[:])

            # ================= PHASE B: the scan =================
            with tc.tile_pool(name="pbp", bufs=1, space="PSUM") as pbp:
                ps_sp = pbp.tile([128, 32], F32)  # pre1 | score_e | pre2 | score_o
                ps_zdot = pbp.tile([128, HC * BL], F32)
                ps_aB = pbp.tile([128, BL], F32)
                ps_zr = pbp.tile([128, 2 * HC * BL], F32)
                ps_hh = pbp.tile([128, HC * BL], F32)
                ps_c = pbp.tile([128, HC * BL], F32)
                ps_sB = pbp.tile([128, BL], F32)

                def bs(c):
                    return slice(c * BL, (c + 1) * BL)

                def step(t_par, xgs):
                    # t_par: timestep parity; xgs: [128, GJ*BL] xg row (fp16)
                    R_ = nc._cta_roles = getattr(nc, "_cta_roles", {})

                    dvm = {}  # role -> instruction (for selective order pins)

                    def tag(inst, role):
                        R_[inst.ins.name] = role
                        dvm[role] = inst
                        return inst
                    z_cur_f, z_cur_h, z_nxt_f, z_nxt_h = (
                        (zA_f, zA_h, zB_f, zB_h) if t_par == 1
                        else (zB_f, zB_h, zA_f, zA_h)
                    )
                    sc = slice(8, 16) if t_par == 0 else slice(24, 32)
                    pe = []  # PE instructions in required issue order

                    def pmm(out, lhsT, rhs, start, stop):
                        i = nc.tensor.matmul(out, lhsT=lhsT, rhs=rhs, start=start,
                                             stop=stop, skip_group_check=True)
                        pe.append(i)
                        return i

                    # xg(z|r) -> ps_zr via identity matmul; the single
                    # start=True writer of ps_zr this step (start wipes the
                    # whole tile's has_written state on this stack)
                    pmm(ps_zr[:], eye_sb[:], xgs[:, 0:8 * BL], True, False)
                    # ODE layer 1 + score matvec (critical: unblocks tanh1)
                    for c in range(HC):
                        pmm(ps_sp[:32, 0:8], w1_sb[:, c], z_cur_h[:, bs(c)],
                            c == 0, c == HC - 1)
                    for c in range(HC):
                        pmm(ps_sp[:1, sc], wkq_sb[:, c], z_cur_h[:, bs(c)],
                            c == 0, c == HC - 1)
                    tag(nc.scalar.activation(h1T[:32, :], ps_sp[:32, 0:8], AF.Tanh,
                                         bias=b1_sb[:, 0:1]), 'tanh1')
                    # GRU z-part (z/r gates), first half
                    for j in range(4):
                        for c in range(HC):
                            pmm(ps_zr[:, j * BL:(j + 1) * BL],
                                rw_sb[:, c, j * 128:(j + 1) * 128],
                                z_cur_h[:, bs(c)], False, False)
                    # ODE layer 2 (h1 ready by now)
                    pmm(ps_sp[:32, 16:24], w2_sb[:], h1T[:], True, True)
                    tag(nc.scalar.activation(h2T[:32, :], ps_sp[:32, 16:24], AF.Tanh,
                                         bias=b2_sb[:, 0:1]), 'tanh2')
                    for j in range(4, 8):
                        for c in range(HC):
                            pmm(ps_zr[:, j * BL:(j + 1) * BL],
                                rw_sb[:, c, j * 128:(j + 1) * 128],
                                z_cur_h[:, bs(c)], False, False)
                    # GRU h2-part for z/r gates (dt*W3@R folded host-side)
                    for j in range(8):
                        pmm(ps_zr[:, j * BL:(j + 1) * BL],
                            w3r_sb[:, j * 128:(j + 1) * 128], h2T[:],
                            False, True)
                    # sigmoid(z|r gates) straight off PSUM
                    tag(nc.scalar.activation(zgrg[:], ps_zr[:], AF.Sigmoid), 'sig_zr')
                    # ODE layer 3 -> zdot psum (for the gate zhat term only)
                    for c in range(HC):
                        pmm(ps_zdot[:, bs(c)], w3_sb[:, c * 128:(c + 1) * 128],
                            h2T[:], True, True)
                    tag(nc.vector.scalar_tensor_tensor(
                        zh_f[:], ps_zdot[:], dt_sb[:, 0:1], z_cur_f[:],
                        OP.mult, OP.add,
                    ), 'zh_f')
                    # GRU candidate block: per-region contiguous groups so the
                    # start-wipe semantics stay correct without a zero-init
                    for j in range(8, 12):
                        for c in range(HC):
                            pmm(ps_hh[:, (j - 8) * BL:(j - 7) * BL],
                                rw_sb[:, c, j * 128:(j + 1) * 128],
                                z_cur_h[:, bs(c)], c == 0, False)
                        pmm(ps_hh[:, (j - 8) * BL:(j - 7) * BL],
                            w3r_sb[:, j * 128:(j + 1) * 128], h2T[:],
                            False, True)
                    # gates: z_new = zg*zhat + (1-zg)*cand.
                    # DVE issue order is pinned: critical chain first, the
                    # alpha/r/s bookkeeping strictly after z_f.
                    tag(nc.vector.tensor_scalar(wgT[:], zgrg[:, 0:4 * BL], -1.0,
                                                1.0, OP.mult, OP.add), 'wg')
                    tag(nc.vector.tensor_tensor(t3[:], zgrg[:, 4 * BL:8 * BL],
                                                ps_hh[:], OP.mult), 't3')
                    tag(nc.vector.tensor_add(t4[:], t3[:],
                                             xgs[:, 8 * BL:12 * BL]), 't4')
                    tag(nc.vector.tensor_tensor(eT[:], zgrg[:, 0:4 * BL],
                                                zh_f[:], OP.mult), 'e')
                    R_[nc.scalar.activation(candT[:], t4[:], AF.Tanh).ins.name] = 'cand'
                    tag(nc.vector.tensor_tensor(uT[:], wgT[:], candT[:],
                                                OP.mult), 'u')
                    tag(nc.vector.tensor_add(z_nxt_h[:], eT[:], uT[:]), 'z_h')
                    tag(nc.vector.tensor_add(z_nxt_f[:], eT[:], uT[:]), 'z_f')
                    # alpha / r / s accumulators (off critical path; z_cur
                    # stays live thanks to the ping-pong)
                    R_[nc.scalar.activation(alphaF[:], ps_sp[:1, sc], AF.Sigmoid,
                                            bias=bia_sb[:, 0:1],
                                            scale=sca_sb[:, 0:1]).ins.name] = 'alpha'
                    tag(nc.vector.tensor_copy(alco[:1, :], alphaF[:]), 'alco')
                    tag(nc.vector.tensor_add(sRow[:], sRow[:], alphaF[:]), 'sRow')
                    pmm(ps_aB[:], ones_sb[:], alco[:], True, True)
                    tag(nc.vector.tensor_tensor(
                        rtmp.rearrange("p (c b) -> p c b", b=BL),
                        z_cur_f.rearrange("p (c b) -> p c b", b=BL),
                        ps_aB[:, None, :].to_broadcast((128, HC, BL)),
                        OP.mult,
                    ), 'rtmp')
                    tag(nc.vector.tensor_add(rT[:], rT[:], rtmp[:]), 'rT')
                    # pin the PE issue order (the scheduler otherwise defers
                    # the tanh-chain matmuls behind all the gru pairs)
                    from bass_rust import add_dep_helper
                    for a, b in zip(pe[1:], pe[:-1]):
                        add_dep_helper(a.ins, b.ins, reason="step pe order")


                def load_S(Sbuf, row0):
                    nc.sync.dma_start(Sbuf[:], xgt[:, ds(row0, 16), :])

                # prologue: steps 1..31
                load_S(S0, 0)
                load_S(S1, 16)
                for t in range(1, 32):
                    if t < 16:
                        step(t % 2, S0[:, t, :])
                    else:
                        step(t % 2, S1[:, t - 16, :])
                    if _dbg and t == 1:
                        nc.sync.dma_start(zdbg1[:], zB_f[:])
                    if _dbg and t == 2:
                        nc.sync.dma_start(zdbg2[:], zA_f[:])
                load_S(S0, 32)
                load_S(S1, 48)
                hint = (mybir.EngineType.PE, mybir.EngineType.DVE,
                        mybir.EngineType.Activation)
                with tc.For_i(1, n_bodies, 1, staggered_reset=True,
                              hint_engines=hint) as iv:
                    for k in range(16):
                        step(k % 2, S0[:, k, :])
                    load_S(S0, iv * 32 + 32)
                    for k in range(16, 32):
                        step(k % 2, S1[:, k - 16, :])
                    load_S(S1, iv * 32 + 48)

                # ================= PHASE C =================
                rh = wp.tile([128, HC * BL], F16)
                nc.vector.tensor_copy(rh[:], rT[:])
                for m in range(HC):
                    for c in range(HC):
                        nc.tensor.matmul(
                            ps_c[:, bs(m)], lhsT=wv_sb[:, c, m * 128:(m + 1) * 128],
                            rhs=rh[:, bs(c)], start=(c == 0), stop=(c == HC - 1),
                        )
                sinv = wp.tile([1, BL], F32)
                nc.vector.reciprocal(sinv[:], sRow[:])
                nc.vector.tensor_copy(alco[:1, :], sinv[:])
                nc.tensor.matmul(ps_sB[:], lhsT=ones_sb[:], rhs=alco[:],
                                 start=True, stop=True)
                sb_sinv = wp.tile([128, BL], F32)
                nc.vector.tensor_copy(sb_sinv[:], ps_sB[:])
                ct = wp.tile([128, HC * BL], F32)
                nc.vector.tensor_tensor(
                    ct.rearrange("p (c b) -> p c b", b=BL),
                    ps_c.rearrange("p (c b) -> p c b", b=BL),
                    sb_sinv[:, None, :].to_broadcast((128, HC, BL)),
                    OP.mult,
                )
                outT = wp.tile([128, HC * BL], F32)
                nc.vector.tensor_add(outT[:], zB_f[:], ct[:])
                nc.sync.dma_start(outt[:], outT[:])

    import os
    if os.environ.get("CTA_NO_WSPLIT") != "1":
        _split_multi_waits(nc)
    return nc


# ---------------------------------------------------------------------------
# Host side
# ---------------------------------------------------------------------------
_PROGRAM_CACHE = {}


def _get_program(n_steps):
    if n_steps not in _PROGRAM_CACHE:
        _PROGRAM_CACHE[n_steps] = build_program(n_steps)
    return _PROGRAM_CACHE[n_steps]


def _chunked(a):
    """[K*128, N] -> [128, K, N] with chunk index second."""
    k = a.shape[0] // 128
    return np.ascontiguousarray(
        a.reshape(k, 128, *a.shape[1:]).transpose(1, 0, *range(2, a.ndim + 1))
    )


def prepare_host(inputs, n_steps=S):
    inp = {k: np.asarray(v) for k, v in inputs.items()}
    f32 = np.float32
    x = inp["x"].astype(f32, copy=False)
    Wp, bp = inp["Wp"].astype(f32), inp["bp"].astype(f32)
    W1, b1 = inp["ode_W1"].astype(f32), inp["ode_b1"].astype(f32)
    W2, b2 = inp["ode_W2"].astype(f32), inp["ode_b2"].astype(f32)
    W3, b3 = inp["ode_W3"].astype(f32), inp["ode_b3"].astype(f32)
    Wq, bq = inp["Wq"].astype(f32), inp["bq"].astype(f32)
    Wk, bk = inp["Wk"].astype(f32), inp["bk"].astype(f32)
    Wv, bv = inp["Wv"].astype(f32), inp["bv"].astype(f32)
    query = inp["query"].astype(f32)
    tsc = np.float64(inp["time_scale"])
    gK, gR, gb = inp["gru_K"].astype(f32), inp["gru_R"].astype(f32), inp["gru_b"].astype(f32)

    dt = f32(np.log1p(np.exp(tsc)))
    inv = f32(1.0 / np.sqrt(H))
    q = (query @ Wq + bq)[0]
    wkqv = Wk @ q
    bkq = f32(bk @ q)
    Wxg = Wp @ gK
    xg_const = bp @ gK + gb[0]
    constv = xg_const.copy()
    constv[:2 * H] += gb[1][:2 * H]
    assert not np.any(b3), "nonzero ode_b3 path not implemented"
    assert not np.any(gb[1][2 * H:]), "nonzero gru hh-bias path not implemented"

    z0 = x[:, 0] @ Wp + bp  # [B, H] exact fp32

    w2p = np.zeros((128, 32), f32); w2p[:32] = W2
    w3p = np.zeros((128, H), f32); w3p[:32] = W3
    w3r = np.zeros((128, G), f32); w3r[:32] = dt * (W3 @ gR)
    onesl = np.zeros((128, 128), f32); onesl[0, :] = 1.0

    shared = {
        "rw": _chunked(gR).astype(np.float16),
        "wxg": _chunked(Wxg).astype(np.float16),
        "wv": _chunked(Wv).astype(np.float16),
        "w1": _chunked(W1).astype(np.float16),
        "wkq": _chunked(wkqv[:, None]).astype(np.float16),
        "w2p": w2p.astype(np.float16),
        "w3p": w3p.astype(np.float16),
        "w3r": w3r.astype(np.float16),
        "eye": np.eye(128, dtype=np.float16),
        "onesl": onesl.astype(np.float16),
        "constj": np.ascontiguousarray(
            constv.reshape(GJ, 128).T).astype(f32),
        "dtcol": np.full((128, 1), dt, f32),
        "sca": np.array([[inv]], f32),
        "bia": np.array([[bkq * inv]], f32),
        "b1t": b1.reshape(32, 1).astype(f32),
        "b2t": b2.reshape(32, 1).astype(f32),
    }

    x16 = x.astype(np.float16)
    in_maps = []
    for ci in range(NCORES):
        xs = x16[ci * BL:(ci + 1) * BL]              # [BL, S, DIN]
        xtc = np.ascontiguousarray(xs.transpose(2, 1, 0))  # [DIN, S, BL]
        z0s = z0[ci * BL:(ci + 1) * BL]              # [BL, H]
        z0tc = np.ascontiguousarray(
            z0s.reshape(BL, HC, 128).transpose(2, 1, 0).reshape(128, HC * BL))
        m = dict(shared)
        m["xt"] = xtc
        m["z0t"] = z0tc.astype(f32)
        in_maps.append(m)
    return in_maps, (Wv, bv)


def assemble_output(results):
    out = np.empty((B, H), np.float32)
    for ci, r in enumerate(results):
        o = r["outt"].reshape(128, HC, BL).transpose(2, 1, 0).reshape(BL, H)
        out[ci * BL:(ci + 1) * BL] = o
    return out


def run(inputs, n_steps=S, **run_kwargs):
    in_maps, _ = prepare_host(inputs, n_steps)
    nc = _get_program(n_steps)
    res = run_bass_kernel_spmd(nc, in_maps, core_ids=list(range(NCORES)),
                               **run_kwargs)
    return assemble_output(res.results), res


def kernel(**inputs):
    out, _ = run(inputs)
    return out

